# revision 1
# baseline (speedup 1.0000x reference)
"""Trainium2 Bass kernel for nn_EnhancedAttentionLayer (segment softmax MLP).

Host contract: kernel(**inputs) takes the FULL unsharded inputs from
setup_inputs() and returns the FULL [N, 1] float32 output.

Strategy
--------
Math:  out = mean_heads( softmax_per_segment( sigmoid( Wa-head of
       LN( relu( relu([x, alpha] @ W1 + b1) @ W2 + b2 ) ) ) ) )

Device layout ("transposed chain", weights stationary as lhsT):
  xT [D, rows]  --W1-->  h1T [H, rows]  --W2-->  h2T [D, rows]
Matmul contracts over the partition dim, so feeding x transposed keeps
every matmul's moving operand at N=512 free-dim (full PE efficiency) with
zero on-device transposes of activations in the main chain.

Folds (host-side precompute):
  - alpha column:  b1_eff = b1 + alpha * W1[D]   (concat removed)
  - LN scale/bias g,b folded into the head projection:
      raw_att[r,a] = rs[r]*P[a,r] - (rs*mu)[r]*s[a] + c1[a]
      P = (g.Wa)^T @ h2T,  s = sum_d g*Wa,  c1 = b@Wa + ba
  - segment max subtraction dropped: z = sigmoid(.) in (0,1), so
    softmax = exp(z)/segsum(exp(z)) is numerically safe without it.

Segment softmax on device: rows are pre-packed on host so every 512-row
supertile is segment-aligned (a segment never straddles supertiles) with
<=127 segments per supertile.  Per 128-row subtile a one-hot S[r, j] =
(j == local_seg[r]) is built with a DVE is_equal; segment sums are
S^T @ E (PE matmul), the gather back is S @ recip(segsum) (PE matmul with
S transposed on PE).

Sharding: 8 cores data-parallel over supertiles (segments never cross
cores).  Weights replicated.  SPMD: one Bass program, per-core inputs.
"""

import sys

sys.path.insert(0, "/opt/trn_rl_repo")

from contextlib import ExitStack

import ml_dtypes
import numpy as np

import concourse.bass as bass
import concourse.tile as tile
from concourse import bacc, bass_utils, mybir

BF16 = mybir.dt.bfloat16
F32 = mybir.dt.float32
AF = mybir.ActivationFunctionType
OP = mybir.AluOpType

D = 512
H = 2048
HEADS = 4
RSUP = 512          # rows per supertile
NSUP = 25           # supertiles per core
NCORES = 8
EPS = 1e-5
KT1 = D // 128      # 4  k-tiles for layer 1
KT2 = H // 128      # 16 k-tiles for layer 2
HC1 = H // 128      # 16 output chunks for layer 1
DC2 = D // 128      # 4  output chunks for layer 2
NSUB = RSUP // 128  # 4  subtiles per supertile


def _build_bass(nsup: int):
    """Build the SPMD single-core Bass program for `nsup` supertiles."""
    nc = bacc.Bacc(
        "TRN2", target_bir_lowering=False, debug=False, enable_asserts=False
    )
    nr = nsup * RSUP

    xt_d = nc.dram_tensor("xt", [D, nr], BF16, kind="ExternalInput").ap()
    cl_d = nc.dram_tensor("cl", [128, nsup * NSUB], F32, kind="ExternalInput").ap()
    w1_d = nc.dram_tensor("w1", [D, H], BF16, kind="ExternalInput").ap()
    b1_d = nc.dram_tensor("b1e", [128, HC1], F32, kind="ExternalInput").ap()
    w2_d = nc.dram_tensor("w2", [H, D], BF16, kind="ExternalInput").ap()
    b2_d = nc.dram_tensor("b2t", [128, DC2], F32, kind="ExternalInput").ap()
    wg_d = nc.dram_tensor("wg", [D, 33], BF16, kind="ExternalInput").ap()
    ns_d = nc.dram_tensor("negs", [1, HEADS], BF16, kind="ExternalInput").ap()
    e1_d = nc.dram_tensor("e1b", [128, HEADS], F32, kind="ExternalInput").ap()
    io_d = nc.dram_tensor("iota", [128, 128], F32, kind="ExternalInput").ap()
    id_d = nc.dram_tensor("ident", [128, 128], F32, kind="ExternalInput").ap()
    out_d = nc.dram_tensor("out", [128, nsup * NSUB], F32, kind="ExternalOutput").ap()

    with tile.TileContext(nc) as tc, ExitStack() as ctx:
        consts = ctx.enter_context(tc.tile_pool(name="consts", bufs=1))
        xp = ctx.enter_context(tc.tile_pool(name="xp", bufs=2))
        hp = ctx.enter_context(tc.tile_pool(name="hp", bufs=2))
        ep = ctx.enter_context(tc.tile_pool(name="ep", bufs=2))
        sp = ctx.enter_context(tc.tile_pool(name="sp", bufs=3))
        pbig = ctx.enter_context(tc.tile_pool(name="pbig", bufs=3, space="PSUM"))
        pstat = ctx.enter_context(tc.tile_pool(name="pstat", bufs=1, space="PSUM"))
        pseg = ctx.enter_context(tc.tile_pool(name="pseg", bufs=2, space="PSUM"))
        psmall = ctx.enter_context(tc.tile_pool(name="psmall", bufs=2, space="PSUM"))

        # ---- constants, loaded once -------------------------------------
        # w1 + b1 + the first supertile's xT go first so layer 1 of
        # supertile 0 can start while the remaining consts stream in.
        w1_sb = consts.tile([128, KT1, H], BF16)
        nc.sync.dma_start(out=w1_sb, in_=w1_d.rearrange("(a p) h -> p a h", p=128))
        b1_sb = consts.tile([128, HC1], F32)
        nc.sync.dma_start(out=b1_sb, in_=b1_d)
        xt_first = xp.tile([128, KT1, RSUP], BF16, tag="xt")
        nc.sync.dma_start(
            out=xt_first,
            in_=xt_d[:, 0:RSUP].rearrange("(a p) c -> p a c", p=128),
        )
        w2_sb = consts.tile([128, KT2, D], BF16)
        nc.sync.dma_start(out=w2_sb, in_=w2_d.rearrange("(a p) d -> p a d", p=128))
        wg_sb = consts.tile([128, KT1, 33], BF16)
        nc.sync.dma_start(out=wg_sb, in_=wg_d.rearrange("(a p) h -> p a h", p=128))
        ns_sb = consts.tile([1, HEADS], BF16)
        nc.sync.dma_start(out=ns_sb, in_=ns_d)
        b2_sb = consts.tile([128, DC2], F32)
        nc.sync.dma_start(out=b2_sb, in_=b2_d)
        e1_sb = consts.tile([128, HEADS], F32)
        nc.sync.dma_start(out=e1_sb, in_=e1_d)
        iota_sb = consts.tile([128, 128], F32)
        nc.sync.dma_start(out=iota_sb, in_=io_d)
        cl_sb = consts.tile([128, nsup * NSUB], F32)
        nc.sync.dma_start(out=cl_sb, in_=cl_d)

        ones_sb = consts.tile([128, 1], BF16)
        nc.vector.memset(ones_sb, 1.0 / D)
        id_sb = consts.tile([128, 128], F32)
        nc.sync.dma_start(out=id_sb, in_=id_d)
        eps_sb = consts.tile([128, 1], F32)
        nc.vector.memset(eps_sb, EPS)

        out_sb = consts.tile([128, nsup * NSUB], F32)

        def emit_late(pl):
            """Sigmoid -> E -> segment sum -> reciprocal -> gather -> out for
            supertile st0.  Emitted one supertile late, right after the next
            supertile's L2 matmuls, so the PE meets only resolved deps."""
            st0, tn4, pt_ev, s_list, st_t0 = pl
            e_t = ep.tile([128, NSUB, HEADS], F32, tag="e", name="e_t")
            seg_ps = pseg.tile([128, HEADS], F32, tag="seg", name="seg_ps")
            for sub in range(NSUB):
                ew = sp.tile([128, HEADS], F32, tag="ew", name="ew")
                nc.scalar.activation(
                    ew, pt_ev[:, sub * HEADS : (sub + 1) * HEADS],
                    AF.Exp, scale=tn4[:, sub : sub + 1],
                )
                nc.vector.tensor_tensor(ew, ew, e1_sb, op=OP.mult)
                nc.vector.tensor_scalar(ew, ew, 1.0, None, op0=OP.add)
                sg = sp.tile([128, HEADS], F32, tag="sg", name="sg")
                nc.vector.reciprocal(sg, ew)
                nc.scalar.activation(e_t[:, sub, :], sg, AF.Exp)
                nc.tensor.matmul(
                    seg_ps, s_list[sub], e_t[:, sub, :],
                    start=(sub == 0), stop=(sub == NSUB - 1),
                )
            segr = sp.tile([128, HEADS], F32, tag="segr", name="segr")
            nc.vector.tensor_scalar(segr, seg_ps, 1e-30, None, op0=OP.add)
            nc.vector.reciprocal(segr, segr)
            for sub in range(NSUB):
                col = st0 * NSUB + sub
                r_ps = psmall.tile([128, HEADS], F32, tag="ps_small", name="r_ps")
                nc.tensor.matmul(
                    r_ps, st_t0[:, sub, :], segr, start=True, stop=True
                )
                nrm = sp.tile([128, HEADS], F32, tag="nrm", name="nrm")
                nc.vector.tensor_tensor(nrm, e_t[:, sub, :], r_ps, op=OP.mult)
                nc.vector.tensor_reduce(
                    out_sb[:, col : col + 1], nrm,
                    axis=mybir.AxisListType.X, op=OP.add,
                )

        pend: list = []
        for st in range(nsup):
            r0 = st * RSUP
            # ---- load x^T for this supertile ----------------------------
            if st == 0:
                xt_t = xt_first
            else:
                xt_t = xp.tile([128, KT1, RSUP], BF16, tag="xt", name="xt_t")
                nc.sync.dma_start(
                    out=xt_t,
                    in_=xt_d[:, r0 : r0 + RSUP].rearrange("(a p) c -> p a c", p=128),
                )

            # ---- layer 1: h1T[hc] = relu(W1[:,hc].T @ xT + b1) ----------
            h1_t = hp.tile([128, HC1, RSUP], BF16, tag="h1")
            for hc in range(HC1):
                ps1 = pbig.tile([128, RSUP], F32, tag="pbig")
                for kt in range(KT1):
                    nc.tensor.matmul(
                        ps1,
                        w1_sb[:, kt, hc * 128 : (hc + 1) * 128],
                        xt_t[:, kt, :],
                        start=(kt == 0),
                        stop=(kt == KT1 - 1),
                    )
                if hc % 8 < 5:
                    nc.scalar.activation(
                        h1_t[:, hc, :], ps1, AF.Relu, bias=b1_sb[:, hc : hc + 1]
                    )
                else:
                    nc.vector.tensor_scalar(
                        h1_t[:, hc, :], ps1, b1_sb[:, hc : hc + 1], 0.0,
                        op0=OP.add, op1=OP.max,
                    )

            # ---- layer 2 + head projection (with mu as 5th column) ------
            h2_t = hp.tile([128, DC2, RSUP], BF16, tag="h2")
            sq_t = hp.tile([128, DC2, RSUP], BF16, tag="sq")
            p_ps = pstat.tile([65, RSUP], F32, tag="p")
            for dc in range(DC2):
                ps2 = pbig.tile([128, RSUP], F32, tag="pbig")
                for kt in range(KT2):
                    nc.tensor.matmul(
                        ps2,
                        w2_sb[:, kt, dc * 128 : (dc + 1) * 128],
                        h1_t[:, kt, :],
                        start=(kt == 0),
                        stop=(kt == KT2 - 1),
                    )
                h2c = h2_t[:, dc, :]
                nc.scalar.activation(h2c, ps2, AF.Relu, bias=b2_sb[:, dc : dc + 1])
                nc.vector.tensor_tensor(sq_t[:, dc, :], h2c, h2c, op=OP.mult)
                nc.tensor.matmul(
                    p_ps[0:33, :], wg_sb[:, dc, :], h2c,
                    start=(dc == 0), stop=(dc == DC2 - 1),
                )

            # msq: tree-sum the squares on DVE, one reduce matmul
            t01_sb = hp.tile([128, RSUP], BF16, tag="t01")
            nc.vector.tensor_tensor(t01_sb, sq_t[:, 0, :], sq_t[:, 1, :], op=OP.add)
            t23_sb = hp.tile([128, RSUP], BF16, tag="t23")
            nc.vector.tensor_tensor(t23_sb, sq_t[:, 2, :], sq_t[:, 3, :], op=OP.add)
            sqs_sb = hp.tile([128, RSUP], BF16, tag="sqs")
            nc.vector.tensor_tensor(sqs_sb, t01_sb, t23_sb, op=OP.add)
            ms_ps = p_ps[64:65, :]
            nc.tensor.matmul(ms_ps, ones_sb, sqs_sb, start=True, stop=True,
                             skip_group_check=True)

            # mu out of P row 32; rank-1 term  (-s) * mu  in its own bank
            mu_sb = sp.tile([1, RSUP], F32, tag="mu_sb")
            nc.vector.tensor_copy(mu_sb, p_ps[32:33, :])
            mu_bf = sp.tile([1, RSUP], BF16, tag="mu_bf")
            nc.vector.tensor_copy(mu_bf, mu_sb)
            sm_ps = psmall.tile([HEADS, RSUP], F32, tag="ps_small")
            nc.tensor.matmul(sm_ps, ns_sb, mu_bf, start=True, stop=True)

            # one-hot S per subtile (no stat dependency; built early so the
            # S^T transposes are PE-ready right after L2)
            s_list = []
            for sub in range(NSUB):
                col = st * NSUB + sub
                s_sb = sp.tile([128, 128], F32, tag="s_sb", bufs=8, name="s_sb")
                nc.vector.tensor_scalar(
                    s_sb, iota_sb, cl_sb[:, col : col + 1], None, op0=OP.is_equal
                )
                s_list.append(s_sb)

            # previous supertile's E/segsum/gather: deps resolved by now
            if pend:
                emit_late(pend.pop(0))

            # ---- LN stats: var on [1,512]; transpose; ln/exp in [128,4] --
            mu2_sb = sp.tile([1, RSUP], F32, tag="mu2_sb")
            nc.vector.tensor_tensor(mu2_sb, mu_sb, mu_sb, op=OP.mult)
            var_sb = sp.tile([1, RSUP], F32, tag="var_sb")
            nc.vector.tensor_tensor(var_sb, ms_ps, mu2_sb, op=OP.subtract)
            vt_ps = psmall.tile([128, NSUB], F32, tag="ps_small")
            for sub in range(NSUB):
                nc.tensor.transpose(
                    vt_ps[:, sub : sub + 1],
                    var_sb[:, sub * 128 : (sub + 1) * 128],
                    id_sb[0:1, 0:1],
                )
            vt_ev = sp.tile([128, NSUB], F32, tag="vt_ev")
            nc.vector.tensor_copy(vt_ev, vt_ps)
            ln4_sb = sp.tile([128, NSUB], F32, tag="ln4_sb")
            nc.scalar.activation(ln4_sb, vt_ev, AF.Ln, bias=eps_sb)
            tn4 = sp.tile([128, NSUB], F32, tag="tn4")
            nc.scalar.activation(tn4, ln4_sb, AF.Exp, scale=-0.5)
            nc.vector.tensor_scalar(tn4, tn4, -1.0, None, op0=OP.mult)

            p_sb = sp.tile([HEADS, RSUP], F32, tag="p_sb")
            nc.vector.tensor_copy(p_sb, p_ps[0:HEADS, :])
            nc.vector.tensor_tensor(p_sb, p_sb, sm_ps, op=OP.add)
            pt_ps = psmall.tile([128, HEADS * NSUB], F32, tag="ps_small")
            for sub in range(NSUB):
                nc.tensor.transpose(
                    pt_ps[:, sub * HEADS : (sub + 1) * HEADS],
                    p_sb[:, sub * 128 : (sub + 1) * 128],
                    id_sb[0:HEADS, 0:HEADS],
                )
            pt_ev = sp.tile([128, HEADS * NSUB], F32, tag="pt_ev")
            nc.vector.tensor_copy(pt_ev, pt_ps)

            st_t = ep.tile([128, NSUB, 128], F32, tag="st")
            for sub in range(NSUB):
                st_ps = psmall.tile([128, 128], F32, tag="ps_small", name="st_ps")
                nc.tensor.transpose(st_ps, s_list[sub], id_sb)
                nc.vector.tensor_copy(st_t[:, sub, :], st_ps)

            pend.append((st, tn4, pt_ev, s_list, st_t))

        emit_late(pend.pop(0))
        nc.sync.dma_start(out=out_d, in_=out_sb)

    nc.finalize()
    return nc


_BUILD_CACHE: dict = {}


def _get_bass(nsup: int):
    if nsup not in _BUILD_CACHE:
        _BUILD_CACHE[nsup] = _build_bass(nsup)
    return _BUILD_CACHE[nsup]


def _host_prep(x, row, alpha, W1, b1, W2, b2, ln_g, ln_b, Wa, ba,
               nsup=NSUP, ncores=NCORES):
    """Pack rows into segment-aligned supertiles, build per-core inputs."""
    bf16 = ml_dtypes.bfloat16
    N = x.shape[0]
    row = np.asarray(row).astype(np.int64)

    # segment runs (row is sorted)
    change = np.flatnonzero(np.diff(row)) + 1
    starts = np.concatenate([[0], change])
    ends = np.concatenate([change, [N]])
    lens = ends - starts
    assert lens.max() <= RSUP, "segment longer than a supertile"

    # greedy pack segments into RSUP-row bins
    bin_of_seg = np.empty(len(starts), np.int64)
    nbins = 0
    cur = 0
    for i, ln in enumerate(lens):
        if cur + ln > RSUP:
            nbins += 1
            cur = 0
        bin_of_seg[i] = nbins
        cur += ln
    nbins += 1
    assert nbins <= ncores * nsup, f"{nbins} bins > capacity {ncores * nsup}"

    nr = nsup * RSUP
    # per-core gather index (source row or -1) and local segment rank
    gidx = np.full((ncores, nr), -1, np.int64)
    cloc = np.full((ncores, nr), 127.0, np.float32)
    seg_rank = np.zeros(len(starts), np.int64)
    # rank of each segment within its bin; position of each segment in bin
    pos_in_bin = np.zeros(len(starts), np.int64)
    cur_bin, cur_pos, cur_rank = -1, 0, 0
    for i in range(len(starts)):
        if bin_of_seg[i] != cur_bin:
            cur_bin, cur_pos, cur_rank = bin_of_seg[i], 0, 0
        pos_in_bin[i] = cur_pos
        seg_rank[i] = cur_rank
        cur_pos += lens[i]
        cur_rank += 1
    assert seg_rank.max() <= 126, "too many segments in one supertile"

    for i in range(len(starts)):
        b = bin_of_seg[i]
        c, stl = divmod(b, nsup)
        base = stl * RSUP + pos_in_bin[i]
        gidx[c, base : base + lens[i]] = np.arange(starts[i], ends[i])
        cloc[c, base : base + lens[i]] = seg_rank[i]

    # x gather + transpose + bf16, per core
    x_ext = np.concatenate([x, np.zeros((1, D), np.float32)], 0).astype(bf16)
    srcs = np.where(gidx < 0, N, gidx)
    xts = []
    for c in range(ncores):
        xc = x_ext[srcs[c]]                       # [nr, D] bf16
        xts.append(np.ascontiguousarray(xc.T))    # [D, nr] bf16

    # cl layout [128, nsup*NSUB]: cl2[p, st*NSUB+sub] = cloc[st*512+sub*128+p]
    cls = [
        np.ascontiguousarray(
            cloc[c].reshape(nsup * NSUB, 128).T
        ).astype(np.float32)
        for c in range(ncores)
    ]

    # replicated weights / constants
    alpha_f = float(np.asarray(alpha).reshape(-1)[0])
    w1 = np.ascontiguousarray(W1[:D]).astype(bf16)                    # [D, H]
    b1_eff = (b1 + alpha_f * W1[D]).astype(np.float32)                # [H]
    b1e = np.ascontiguousarray(b1_eff.reshape(HC1, 128).T)            # [128,16]
    w2 = W2.astype(bf16)                                              # [H, D]
    b2t = np.ascontiguousarray(b2.astype(np.float32).reshape(DC2, 128).T)
    wg_f = (ln_g[:, None] * Wa).astype(np.float32)                    # [D, 4]
    wg = np.concatenate(
        [wg_f, np.zeros((D, 28), np.float32),
         np.full((D, 1), 1.0 / D, np.float32)], 1
    ).astype(bf16)                                                    # [D, 33]
    s_vec = wg_f.sum(0).astype(np.float32)                            # [4]
    c1 = (ln_b @ Wa + ba).astype(np.float32)                          # [4]
    negs = (-s_vec).reshape(1, HEADS).astype(bf16)
    e1b = np.broadcast_to(np.exp(-c1), (128, HEADS)).astype(np.float32).copy()
    iota = np.broadcast_to(
        np.arange(128, dtype=np.float32), (128, 128)
    ).copy()
    ident = np.eye(128, dtype=np.float32)

    in_maps = []
    for c in range(ncores):
        in_maps.append({
            "xt": xts[c], "cl": cls[c], "w1": w1, "b1e": b1e, "w2": w2,
            "b2t": b2t, "wg": wg, "negs": negs, "e1b": e1b, "iota": iota,
            "ident": ident,
        })
    return in_maps, gidx


def _unshard(results, gidx, N):
    out = np.zeros((N, 1), np.float32)
    for c, res in enumerate(results):
        vals = np.ascontiguousarray(res["out"].T).reshape(-1, NSUB, 128)
        vals = vals.reshape(-1)  # position order (st, sub, p)
        m = gidx[c] >= 0
        out[gidx[c][m], 0] = vals[m]
    return out * (1.0 / HEADS)


def kernel(x, row, alpha, W1, b1, W2, b2, ln_g, ln_b, Wa, ba, **_kw):
    x = np.asarray(x, np.float32)
    in_maps, gidx = _host_prep(
        x, row, alpha,
        np.asarray(W1, np.float32), np.asarray(b1, np.float32),
        np.asarray(W2, np.float32), np.asarray(b2, np.float32),
        np.asarray(ln_g, np.float32), np.asarray(ln_b, np.float32),
        np.asarray(Wa, np.float32), np.asarray(ba, np.float32),
    )
    nc = _get_bass(NSUP)
    res = bass_utils.run_bass_kernel_spmd(
        nc, in_maps, core_ids=list(range(NCORES))
    )
    return _unshard(res.results, gidx, x.shape[0])



# revision 8
# speedup vs baseline: 1.1582x; 1.1582x over previous
"""Trainium2 Bass kernel for nn_EnhancedAttentionLayer (segment softmax MLP).

Host contract: kernel(**inputs) takes the FULL unsharded inputs from
setup_inputs() and returns the FULL [N, 1] float32 output.

Strategy
--------
Math:  out = mean_heads( softmax_per_segment( sigmoid( Wa-head of
       LN( relu( relu([x, alpha] @ W1 + b1) @ W2 + b2 ) ) ) ) )

Device layout ("transposed chain", weights stationary as lhsT):
  xT [D, rows]  --W1-->  h1T [H, rows]  --W2-->  h2T [D, rows]
Matmul contracts over the partition dim, so feeding x transposed keeps
every matmul's moving operand at N=512 free-dim (full PE efficiency) with
zero on-device transposes of activations in the main chain.

fp8 DoubleRow: the two big GEMMs can run in fp8e4 (TRN E4M3, max 240)
with perf_mode=DoubleRow, which packs 2 k-subtiles per matmul (2 fp8
MACs/cell/cycle) for ~1.8x PE throughput.  Weights are pre-scaled by a
power of two (W1*8, W2*16) so their uniform(-1/sqrt(K)) entries escape
the e4m3 subnormal range; the scale is divided back out in the (free)
activation-scale of the following relu.  h1 is stored scaled by 8 (no
extra op) and the combined 1/128 lands in L2's activation scale.

Folds (host-side precompute):
  - alpha column:  b1_eff = b1 + alpha * W1[D]   (concat removed)
  - LN scale/bias g,b folded into the head projection; the rank-1
    -s*mu LN term is folded into the projection weights themselves:
      raw_att[r,a] = rs[r] * Pc[a,r] + c1[a],
      Pc = (g.Wa - s/D)^T @ h2T,  s = sum_d g*Wa,  c1 = b@Wa + ba
    wg is stored NEGATED so exp(-raw) = exp(rs * (-Pc)) * exp(-c1)
    needs no negation of rs on device.  wg col 4 = +1/D gives mu.
  - segment max subtraction dropped: z = sigmoid(.) in (0,1), so
    softmax = exp(z)/segsum(exp(z)) is numerically safe without it.

Segment softmax on device: rows are pre-packed on host so every 512-row
supertile is segment-aligned (a segment never straddles supertiles) with
<=127 segments per supertile.  Per 128-row subtile the one-hot S[r, j] =
(j == local_seg[r]) and its transpose are built on HOST and DMAd in
(bf16); segment sums are S^T @ E (PE matmul), the gather back is
S @ recip(segsum) (PE matmul with the DMAd S^T as stationary).

Sharding: 8 cores data-parallel over supertiles (segments never cross
cores).  Weights replicated.  SPMD: one Bass program, per-core inputs.
"""

import sys

sys.path.insert(0, "/opt/trn_rl_repo")

from contextlib import ExitStack

import ml_dtypes
import numpy as np

import concourse.bass as bass
import concourse.tile as tile
from concourse import bacc, bass_utils, mybir

BF16 = mybir.dt.bfloat16
FP8 = mybir.dt.float8e4
F32 = mybir.dt.float32
AF = mybir.ActivationFunctionType
OP = mybir.AluOpType
DR = mybir.MatmulPerfMode.DoubleRow

D = 512
H = 2048
HEADS = 4
RSUP = 512          # rows per supertile
NSUP = 25           # supertiles per core
NCORES = 8
EPS = 1e-5
KT1 = D // 128      # 4  k-tiles for layer 1
KT2 = H // 128      # 16 k-tiles for layer 2
HC1 = H // 128      # 16 output chunks for layer 1
DC2 = D // 128      # 4  output chunks for layer 2
NSUB = RSUP // 128  # 4  subtiles per supertile
WGC = 33            # head-projection width (cols 0-3 heads, col 32 mu)

FP8_L1 = True       # layer 1 in fp8 DoubleRow
FP8_L2 = False      # layer 2 in fp8 DoubleRow
SC1 = 8.0 if FP8_L1 else 1.0      # W1 pre-scale (h1 stored at SC1*h1)
SC2 = 16.0 if FP8_L2 else 1.0     # W2 pre-scale
SCALE2 = 1.0 / (SC1 * SC2)        # L2 activation scale


def _build_bass(nsup: int, fp8_l1: bool, fp8_l2: bool):
    """Build the SPMD single-core Bass program for `nsup` supertiles."""
    nc = bacc.Bacc(
        "TRN2", target_bir_lowering=False, debug=False, enable_asserts=False
    )
    nr = nsup * RSUP
    dt1 = FP8 if fp8_l1 else BF16   # xt / w1 dtype
    dt2 = FP8 if fp8_l2 else BF16   # h1 / w2 dtype
    scale2 = 1.0 / ((8.0 if fp8_l1 else 1.0) * (16.0 if fp8_l2 else 1.0))

    xt_d = nc.dram_tensor("xt", [D, nr], dt1, kind="ExternalInput").ap()
    w1_d = nc.dram_tensor("w1", [D, H], dt1, kind="ExternalInput").ap()
    b1_d = nc.dram_tensor("b1e", [128, HC1], F32, kind="ExternalInput").ap()
    w2_d = nc.dram_tensor("w2", [H, D], dt2, kind="ExternalInput").ap()
    b2_d = nc.dram_tensor("b2t", [128, DC2], F32, kind="ExternalInput").ap()
    wg_d = nc.dram_tensor("wg", [D, WGC], BF16, kind="ExternalInput").ap()
    e1_d = nc.dram_tensor("e1b", [128, HEADS], F32, kind="ExternalInput").ap()
    id_d = nc.dram_tensor("ident", [33, 33], F32, kind="ExternalInput").ap()
    s_d = nc.dram_tensor(
        "sone", [128, nsup, NSUB, 128], BF16, kind="ExternalInput"
    ).ap()
    st_d = nc.dram_tensor(
        "sonet", [128, nsup, NSUB, 128], F32, kind="ExternalInput"
    ).ap()
    out_d = nc.dram_tensor("out", [128, nsup * NSUB], F32, kind="ExternalOutput").ap()

    with tile.TileContext(nc) as tc, ExitStack() as ctx:
        consts = ctx.enter_context(tc.tile_pool(name="consts", bufs=1))
        xp = ctx.enter_context(tc.tile_pool(name="xp", bufs=2))
        hp = ctx.enter_context(tc.tile_pool(name="hp", bufs=2))
        ep = ctx.enter_context(tc.tile_pool(name="ep", bufs=2))
        sp = ctx.enter_context(tc.tile_pool(name="sp", bufs=3))
        sdma = ctx.enter_context(tc.tile_pool(name="sdma", bufs=3))
        pbig = ctx.enter_context(tc.tile_pool(name="pbig", bufs=3, space="PSUM"))
        pstat = ctx.enter_context(tc.tile_pool(name="pstat", bufs=1, space="PSUM"))
        pseg = ctx.enter_context(tc.tile_pool(name="pseg", bufs=2, space="PSUM"))
        psmall = ctx.enter_context(tc.tile_pool(name="psmall", bufs=2, space="PSUM"))

        # ---- constants, loaded once -------------------------------------
        # w1 + b1 + the first supertile's xT go first so layer 1 of
        # supertile 0 can start while the remaining consts stream in.
        w1_sb = consts.tile([128, KT1, H], dt1)
        nc.sync.dma_start(out=w1_sb, in_=w1_d.rearrange("(a p) h -> p a h", p=128))
        b1_sb = consts.tile([128, HC1], F32)
        nc.sync.dma_start(out=b1_sb, in_=b1_d)
        xt_first = xp.tile([128, KT1, RSUP], dt1, tag="xt")
        nc.sync.dma_start(
            out=xt_first,
            in_=xt_d[:, 0:RSUP].rearrange("(a p) c -> p a c", p=128),
        )
        w2_sb = consts.tile([128, KT2, D], dt2)
        nc.sync.dma_start(out=w2_sb, in_=w2_d.rearrange("(a p) d -> p a d", p=128))
        wg_sb = consts.tile([128, KT1, WGC], BF16)
        nc.sync.dma_start(out=wg_sb, in_=wg_d.rearrange("(a p) h -> p a h", p=128))
        b2_sb = consts.tile([128, DC2], F32)
        nc.sync.dma_start(out=b2_sb, in_=b2_d)
        e1_sb = consts.tile([128, HEADS], F32)
        nc.sync.dma_start(out=e1_sb, in_=e1_d)
        id_sb = consts.tile([33, 33], F32)
        nc.sync.dma_start(out=id_sb, in_=id_d)

        ones_sb = consts.tile([128, 1], BF16)
        nc.vector.memset(ones_sb, 1.0 / D)
        eps_sb = consts.tile([128, 1], F32)
        nc.vector.memset(eps_sb, EPS)

        out_sb = consts.tile([128, nsup * NSUB], F32)

        def emit_late(pl):
            """Sigmoid -> E -> segment sum -> reciprocal -> gather -> out for
            supertile st0.  Emitted one supertile late, right after the next
            supertile's L2 matmuls, so the PE meets only resolved deps."""
            st0, rs4, pt_ev, s_t0, st_t0 = pl
            e_t = ep.tile([128, NSUB, HEADS], F32, tag="e", name="e_t")
            e_b = ep.tile([128, NSUB, HEADS], BF16, tag="eb", name="e_b")
            seg_ps = pseg.tile([128, HEADS], F32, tag="seg", name="seg_ps")
            for sub in range(NSUB):
                ew = sp.tile([128, HEADS], F32, tag="ew", name="ew")
                nc.scalar.activation(
                    ew, pt_ev[:, sub, 0:HEADS],
                    AF.Exp, scale=rs4[:, sub : sub + 1],
                )
                nc.vector.tensor_tensor(ew, ew, e1_sb, op=OP.mult)
                nc.vector.tensor_scalar(ew, ew, 1.0, None, op0=OP.add)
                sg = sp.tile([128, HEADS], F32, tag="sg", name="sg")
                nc.vector.reciprocal(sg, ew)
                nc.scalar.activation(e_t[:, sub, :], sg, AF.Exp)
                nc.vector.tensor_copy(e_b[:, sub, :], e_t[:, sub, :])
                nc.tensor.matmul(
                    seg_ps, s_t0[:, sub, :], e_b[:, sub, :],
                    start=(sub == 0), stop=(sub == NSUB - 1),
                )
            segr = sp.tile([128, HEADS], F32, tag="segr", name="segr")
            nc.vector.tensor_scalar(segr, seg_ps, 1e-30, None, op0=OP.add)
            nc.vector.reciprocal(segr, segr)
            for sub in range(NSUB):
                col = st0 * NSUB + sub
                r_ps = psmall.tile([128, HEADS], F32, tag="ps_small", name="r_ps")
                nc.tensor.matmul(
                    r_ps, st_t0[:, sub, :], segr, start=True, stop=True
                )
                nrm = sp.tile([128, HEADS], F32, tag="nrm", name="nrm")
                nc.vector.tensor_tensor(nrm, e_t[:, sub, :], r_ps, op=OP.mult)
                nc.vector.tensor_reduce(
                    out_sb[:, col : col + 1], nrm,
                    axis=mybir.AxisListType.X, op=OP.add,
                )

        pend: list = []
        for st in range(nsup):
            r0 = st * RSUP
            # ---- load x^T + one-hots for this supertile -----------------
            if st == 0:
                xt_t = xt_first
            else:
                xt_t = xp.tile([128, KT1, RSUP], dt1, tag="xt", name="xt_t")
                nc.sync.dma_start(
                    out=xt_t,
                    in_=xt_d[:, r0 : r0 + RSUP].rearrange("(a p) c -> p a c", p=128),
                )
            s_sb = sdma.tile([128, NSUB, 128], BF16, tag="s", name="s_sb")
            nc.sync.dma_start(out=s_sb, in_=s_d[:, st])
            st_sb = sdma.tile([128, NSUB, 128], F32, tag="stp", name="st_sb")
            nc.sync.dma_start(out=st_sb, in_=st_d[:, st])

            # ---- layer 1: h1T[hc] = relu(W1[:,hc].T @ xT + b1) ----------
            h1_t = hp.tile([128, KT2, RSUP], dt2, tag="h1")
            for hc in range(HC1):
                ps1 = pbig.tile([128, RSUP], F32, tag="pbig")
                if fp8_l1:
                    for kt in range(KT1 // 2):
                        nc.tensor.matmul(
                            ps1,
                            w1_sb[:, 2 * kt : 2 * kt + 2, hc * 128 : (hc + 1) * 128],
                            xt_t[:, 2 * kt : 2 * kt + 2, :],
                            start=(kt == 0),
                            stop=(kt == KT1 // 2 - 1),
                            perf_mode=DR,
                        )
                else:
                    for kt in range(KT1):
                        nc.tensor.matmul(
                            ps1,
                            w1_sb[:, kt, hc * 128 : (hc + 1) * 128],
                            xt_t[:, kt, :],
                            start=(kt == 0),
                            stop=(kt == KT1 - 1),
                        )
                if hc % 8 < 5:
                    nc.scalar.activation(
                        h1_t[:, hc, :], ps1, AF.Relu, bias=b1_sb[:, hc : hc + 1]
                    )
                else:
                    nc.vector.tensor_scalar(
                        h1_t[:, hc, :], ps1, b1_sb[:, hc : hc + 1], 0.0,
                        op0=OP.add, op1=OP.max,
                    )

            # ---- layer 2 + head projection (with mu as 5th column) ------
            h2_t = hp.tile([128, DC2, RSUP], BF16, tag="h2")
            sq_t = hp.tile([128, DC2, RSUP], BF16, tag="sq")
            p_ps = pstat.tile([65, RSUP], F32, tag="p")
            for dc in range(DC2):
                ps2 = pbig.tile([128, RSUP], F32, tag="pbig")
                if fp8_l2:
                    for kt in range(KT2 // 2):
                        nc.tensor.matmul(
                            ps2,
                            w2_sb[:, 2 * kt : 2 * kt + 2, dc * 128 : (dc + 1) * 128],
                            h1_t[:, 2 * kt : 2 * kt + 2, :],
                            start=(kt == 0),
                            stop=(kt == KT2 // 2 - 1),
                            perf_mode=DR,
                        )
                else:
                    for kt in range(KT2):
                        nc.tensor.matmul(
                            ps2,
                            w2_sb[:, kt, dc * 128 : (dc + 1) * 128],
                            h1_t[:, kt, :],
                            start=(kt == 0),
                            stop=(kt == KT2 - 1),
                        )
                h2c = h2_t[:, dc, :]
                nc.scalar.activation(
                    h2c, ps2, AF.Relu, bias=b2_sb[:, dc : dc + 1], scale=scale2
                )
                nc.vector.tensor_tensor(sq_t[:, dc, :], h2c, h2c, op=OP.mult)
                nc.tensor.matmul(
                    p_ps[0:WGC, :], wg_sb[:, dc, :], h2c,
                    start=(dc == 0), stop=(dc == DC2 - 1),
                )

            # msq: tree-sum the squares on DVE, one reduce matmul
            t01_sb = hp.tile([128, RSUP], BF16, tag="t01")
            nc.vector.tensor_tensor(t01_sb, sq_t[:, 0, :], sq_t[:, 1, :], op=OP.add)
            t23_sb = hp.tile([128, RSUP], BF16, tag="t23")
            nc.vector.tensor_tensor(t23_sb, sq_t[:, 2, :], sq_t[:, 3, :], op=OP.add)
            sqs_sb = hp.tile([128, RSUP], BF16, tag="sqs")
            nc.vector.tensor_tensor(sqs_sb, t01_sb, t23_sb, op=OP.add)
            ms_ps = p_ps[64:65, :]
            nc.tensor.matmul(ms_ps, ones_sb, sqs_sb, start=True, stop=True,
                             skip_group_check=True)

            # previous supertile's E/segsum/gather: deps resolved by now
            if pend:
                emit_late(pend.pop(0))

            # ---- stats: [33,512] = [-Pc(4); ...; var@32]; 4 transposes ---
            pv_sb = sp.tile([33, RSUP], F32, tag="pv_sb")
            nc.vector.tensor_copy(pv_sb[0:4, :], p_ps[0:4, :])
            mu_sb = sp.tile([1, RSUP], F32, tag="mu_sb")
            nc.vector.tensor_copy(mu_sb, p_ps[32:33, :])
            mu2_sb = sp.tile([1, RSUP], F32, tag="mu2_sb")
            nc.vector.tensor_tensor(mu2_sb, mu_sb, mu_sb, op=OP.mult)
            nc.vector.tensor_tensor(pv_sb[32:33, :], ms_ps, mu2_sb, op=OP.subtract)

            pt_ps = psmall.tile([128, NSUB, 33], F32, tag="ps_small")
            for sub in range(NSUB):
                nc.tensor.transpose(
                    pt_ps[:, sub, :],
                    pv_sb[:, sub * 128 : (sub + 1) * 128],
                    id_sb,
                )
            pt_ev = sp.tile([128, NSUB, 33], F32, tag="pt_ev")
            nc.vector.tensor_copy(pt_ev, pt_ps)

            # rs = 1/std via exp(-0.5*ln(var+eps))  (Rsqrt AF is banned)
            ln4_sb = sp.tile([128, NSUB], F32, tag="ln4_sb")
            nc.scalar.activation(ln4_sb, pt_ev[:, :, 32], AF.Ln, bias=eps_sb)
            rs4 = sp.tile([128, NSUB], F32, tag="rs4")
            nc.scalar.activation(rs4, ln4_sb, AF.Exp, scale=-0.5)

            pend.append((st, rs4, pt_ev, s_sb, st_sb))

        emit_late(pend.pop(0))
        nc.sync.dma_start(out=out_d, in_=out_sb)

    nc.finalize()
    return nc


_BUILD_CACHE: dict = {}


def _get_bass(nsup: int, fp8_l1: bool = FP8_L1, fp8_l2: bool = FP8_L2):
    key = (nsup, fp8_l1, fp8_l2)
    if key not in _BUILD_CACHE:
        _BUILD_CACHE[key] = _build_bass(nsup, fp8_l1, fp8_l2)
    return _BUILD_CACHE[key]


def _host_prep(x, row, alpha, W1, b1, W2, b2, ln_g, ln_b, Wa, ba,
               nsup=NSUP, ncores=NCORES):
    """Pack rows into segment-aligned supertiles, build per-core inputs."""
    bf16 = ml_dtypes.bfloat16
    fp8 = ml_dtypes.float8_e4m3
    np1 = fp8 if FP8_L1 else bf16
    np2 = fp8 if FP8_L2 else bf16
    N = x.shape[0]
    row = np.asarray(row).astype(np.int64)

    # segment runs (row is sorted)
    change = np.flatnonzero(np.diff(row)) + 1
    starts = np.concatenate([[0], change])
    ends = np.concatenate([change, [N]])
    lens = ends - starts
    assert lens.max() <= RSUP, "segment longer than a supertile"

    # greedy pack segments into RSUP-row bins
    bin_of_seg = np.empty(len(starts), np.int64)
    nbins = 0
    cur = 0
    for i, ln in enumerate(lens):
        if cur + ln > RSUP:
            nbins += 1
            cur = 0
        bin_of_seg[i] = nbins
        cur += ln
    nbins += 1
    assert nbins <= ncores * nsup, f"{nbins} bins > capacity {ncores * nsup}"

    nr = nsup * RSUP
    # per-core gather index (source row or -1) and local segment rank
    gidx = np.full((ncores, nr), -1, np.int64)
    cloc = np.full((ncores, nr), 127, np.int64)
    seg_rank = np.zeros(len(starts), np.int64)
    # rank of each segment within its bin; position of each segment in bin
    pos_in_bin = np.zeros(len(starts), np.int64)
    cur_bin, cur_pos, cur_rank = -1, 0, 0
    for i in range(len(starts)):
        if bin_of_seg[i] != cur_bin:
            cur_bin, cur_pos, cur_rank = bin_of_seg[i], 0, 0
        pos_in_bin[i] = cur_pos
        seg_rank[i] = cur_rank
        cur_pos += lens[i]
        cur_rank += 1
    assert seg_rank.max() <= 126, "too many segments in one supertile"

    for i in range(len(starts)):
        b = bin_of_seg[i]
        c, stl = divmod(b, nsup)
        base = stl * RSUP + pos_in_bin[i]
        gidx[c, base : base + lens[i]] = np.arange(starts[i], ends[i])
        cloc[c, base : base + lens[i]] = seg_rank[i]

    # x gather + transpose, per core
    x_ext = np.concatenate([x, np.zeros((1, D), np.float32)], 0).astype(np1)
    srcs = np.where(gidx < 0, N, gidx)
    xts = []
    for c in range(ncores):
        xc = x_ext[srcs[c]]                       # [nr, D]
        xts.append(np.ascontiguousarray(xc.T))    # [D, nr]

    # one-hot S [r, j] and S^T [j, r] per subtile, built on host.
    # layout [128, nsup, NSUB, 128]: s[p, st, sub, j] refers to row
    # st*512 + sub*128 + p, local segment id j.
    jj = np.arange(128)
    ss, sts = [], []
    for c in range(ncores):
        cl = cloc[c].reshape(nsup, NSUB, 128)     # [st, sub, p]
        s4 = (cl[:, :, :, None] == jj[None, None, None, :])   # [st,sub,p,j]
        ss.append(np.ascontiguousarray(
            s4.transpose(2, 0, 1, 3)).astype(bf16))           # [p,st,sub,j]
        sts.append(np.ascontiguousarray(
            s4.transpose(3, 0, 1, 2)).astype(np.float32))     # [j,st,sub,p]

    # replicated weights / constants
    alpha_f = float(np.asarray(alpha).reshape(-1)[0])
    sc1 = 8.0 if FP8_L1 else 1.0
    sc2 = 16.0 if FP8_L2 else 1.0
    w1 = np.ascontiguousarray(W1[:D] * sc1).astype(np1)               # [D, H]
    b1_eff = (sc1 * (b1 + alpha_f * W1[D])).astype(np.float32)        # [H]
    b1e = np.ascontiguousarray(b1_eff.reshape(HC1, 128).T)            # [128,16]
    w2 = (W2 * sc2).astype(np2)                                       # [H, D]
    b2t = np.ascontiguousarray(b2.astype(np.float32).reshape(DC2, 128).T)
    wg_f = (ln_g[:, None] * Wa).astype(np.float32)                    # [D, 4]
    s_vec = wg_f.sum(0, keepdims=True)                                # [1, 4]
    wg = np.zeros((D, WGC), np.float32)
    wg[:, 0:HEADS] = -(wg_f - s_vec / D)
    wg[:, 32] = 1.0 / D
    wg = wg.astype(bf16)                                              # [D, 16]
    c1 = (ln_b @ Wa + ba).astype(np.float32)                          # [4]
    e1b = np.broadcast_to(np.exp(-c1), (128, HEADS)).astype(np.float32).copy()
    ident = np.eye(33, dtype=np.float32)

    in_maps = []
    for c in range(ncores):
        in_maps.append({
            "xt": xts[c], "w1": w1, "b1e": b1e, "w2": w2,
            "b2t": b2t, "wg": wg, "e1b": e1b, "ident": ident,
            "sone": ss[c], "sonet": sts[c],
        })
    return in_maps, gidx


def _unshard(results, gidx, N):
    out = np.zeros((N, 1), np.float32)
    for c, res in enumerate(results):
        vals = np.ascontiguousarray(res["out"].T).reshape(-1, NSUB, 128)
        vals = vals.reshape(-1)  # position order (st, sub, p)
        m = gidx[c] >= 0
        out[gidx[c][m], 0] = vals[m]
    return out * (1.0 / HEADS)


def kernel(x, row, alpha, W1, b1, W2, b2, ln_g, ln_b, Wa, ba, **_kw):
    x = np.asarray(x, np.float32)
    in_maps, gidx = _host_prep(
        x, row, alpha,
        np.asarray(W1, np.float32), np.asarray(b1, np.float32),
        np.asarray(W2, np.float32), np.asarray(b2, np.float32),
        np.asarray(ln_g, np.float32), np.asarray(ln_b, np.float32),
        np.asarray(Wa, np.float32), np.asarray(ba, np.float32),
    )
    nc = _get_bass(NSUP)
    res = bass_utils.run_bass_kernel_spmd(
        nc, in_maps, core_ids=list(range(NCORES))
    )
    return _unshard(res.results, gidx, x.shape[0])


# revision 11
# speedup vs baseline: 1.1774x; 1.0165x over previous
"""Trainium2 Bass kernel for nn_EnhancedAttentionLayer (segment softmax MLP).

Host contract: kernel(**inputs) takes the FULL unsharded inputs from
setup_inputs() and returns the FULL [N, 1] float32 output.

Strategy
--------
Math:  out = mean_heads( softmax_per_segment( sigmoid( Wa-head of
       LN( relu( relu([x, alpha] @ W1 + b1) @ W2 + b2 ) ) ) ) )

Device layout ("transposed chain", weights stationary as lhsT):
  xT [D, rows]  --W1-->  h1T [H, rows]  --W2-->  h2T [D, rows]
Matmul contracts over the partition dim, so feeding x transposed keeps
every matmul's moving operand at N=512 free-dim (full PE efficiency) with
zero on-device transposes of activations in the main chain.

fp8 DoubleRow: the two big GEMMs can run in fp8e4 (TRN E4M3, max 240)
with perf_mode=DoubleRow, which packs 2 k-subtiles per matmul (2 fp8
MACs/cell/cycle) for ~1.8x PE throughput.  Weights are pre-scaled by a
power of two (W1*8, W2*16) so their uniform(-1/sqrt(K)) entries escape
the e4m3 subnormal range; the scale is divided back out in the (free)
activation-scale of the following relu.  h1 is stored scaled by 8 (no
extra op) and the combined 1/128 lands in L2's activation scale.

Folds (host-side precompute):
  - alpha column:  b1_eff = b1 + alpha * W1[D]   (concat removed)
  - LN scale/bias g,b folded into the head projection; the rank-1
    -s*mu LN term is folded into the projection weights themselves:
      raw_att[r,a] = rs[r] * Pc[a,r] + c1[a],
      Pc = (g.Wa - s/D)^T @ h2T,  s = sum_d g*Wa,  c1 = b@Wa + ba
    wg is stored NEGATED so exp(-raw) = exp(rs * (-Pc)) * exp(-c1)
    needs no negation of rs on device.  wg col 4 = +1/D gives mu.
  - segment max subtraction dropped: z = sigmoid(.) in (0,1), so
    softmax = exp(z)/segsum(exp(z)) is numerically safe without it.

Segment softmax on device: rows are pre-packed on host so every 512-row
supertile is segment-aligned (a segment never straddles supertiles) with
<=127 segments per supertile.  Per 128-row subtile the one-hot S[r, j] =
(j == local_seg[r]) and its transpose are built on HOST and DMAd in
(bf16); segment sums are S^T @ E (PE matmul), the gather back is
S @ recip(segsum) (PE matmul with the DMAd S^T as stationary).

Sharding: 8 cores data-parallel over supertiles (segments never cross
cores).  Weights replicated.  SPMD: one Bass program, per-core inputs.
"""

import sys

sys.path.insert(0, "/opt/trn_rl_repo")

from contextlib import ExitStack

import ml_dtypes
import numpy as np

import concourse.bass as bass
import concourse.tile as tile
from concourse import bacc, bass_utils, mybir

BF16 = mybir.dt.bfloat16
FP8 = mybir.dt.float8e4
F32 = mybir.dt.float32
AF = mybir.ActivationFunctionType
OP = mybir.AluOpType
DR = mybir.MatmulPerfMode.DoubleRow

D = 512
H = 2048
HEADS = 4
RSUP = 512          # rows per supertile
NSUP = 25           # supertiles per core
NCORES = 8
EPS = 1e-5
KT1 = D // 128      # 4  k-tiles for layer 1
KT2 = H // 128      # 16 k-tiles for layer 2
HC1 = H // 128      # 16 output chunks for layer 1
DC2 = D // 128      # 4  output chunks for layer 2
NSUB = RSUP // 128  # 4  subtiles per supertile
WGC = 33            # head-projection width (cols 0-3 heads, col 32 mu)

FP8_L1 = True       # layer 1 in fp8 DoubleRow
FP8_L2 = False      # layer 2 in fp8 DoubleRow
SC1 = 8.0 if FP8_L1 else 1.0      # W1 pre-scale (h1 stored at SC1*h1)
SC2 = 16.0 if FP8_L2 else 1.0     # W2 pre-scale
SCALE2 = 1.0 / (SC1 * SC2)        # L2 activation scale


def _build_bass(nsup: int, fp8_l1: bool, fp8_l2: bool):
    """Build the SPMD single-core Bass program for `nsup` supertiles."""
    nc = bacc.Bacc(
        "TRN2", target_bir_lowering=False, debug=False, enable_asserts=False
    )
    nr = nsup * RSUP
    dt1 = FP8 if fp8_l1 else BF16   # xt / w1 dtype
    dt2 = FP8 if fp8_l2 else BF16   # h1 / w2 dtype
    scale2 = 1.0 / ((8.0 if fp8_l1 else 1.0) * (16.0 if fp8_l2 else 1.0))

    xt_d = nc.dram_tensor("xt", [D, nr], dt1, kind="ExternalInput").ap()
    w1_d = nc.dram_tensor("w1", [D, H], dt1, kind="ExternalInput").ap()
    b1_d = nc.dram_tensor("b1e", [128, HC1], F32, kind="ExternalInput").ap()
    w2_d = nc.dram_tensor("w2", [H, D], dt2, kind="ExternalInput").ap()
    b2_d = nc.dram_tensor("b2t", [128, DC2], F32, kind="ExternalInput").ap()
    wg_d = nc.dram_tensor("wg", [D, WGC], BF16, kind="ExternalInput").ap()
    e1_d = nc.dram_tensor("e1b", [128, HEADS], F32, kind="ExternalInput").ap()
    id_d = nc.dram_tensor("ident", [65, 65], F32, kind="ExternalInput").ap()
    s_d = nc.dram_tensor(
        "sone", [128, nsup, NSUB, 128], BF16, kind="ExternalInput"
    ).ap()
    st_d = nc.dram_tensor(
        "sonet", [128, nsup, NSUB, 128], F32, kind="ExternalInput"
    ).ap()
    out_d = nc.dram_tensor("out", [128, nsup * NSUB], F32, kind="ExternalOutput").ap()

    with tile.TileContext(nc) as tc, ExitStack() as ctx:
        consts = ctx.enter_context(tc.tile_pool(name="consts", bufs=1))
        xp = ctx.enter_context(tc.tile_pool(name="xp", bufs=2))
        hp = ctx.enter_context(tc.tile_pool(name="hp", bufs=2))
        ep = ctx.enter_context(tc.tile_pool(name="ep", bufs=2))
        sp = ctx.enter_context(tc.tile_pool(name="sp", bufs=3))
        sdma = ctx.enter_context(tc.tile_pool(name="sdma", bufs=3))
        pbig = ctx.enter_context(tc.tile_pool(name="pbig", bufs=3, space="PSUM"))
        pstat = ctx.enter_context(tc.tile_pool(name="pstat", bufs=1, space="PSUM"))
        pseg = ctx.enter_context(tc.tile_pool(name="pseg", bufs=1, space="PSUM"))
        psmall = ctx.enter_context(tc.tile_pool(name="psmall", bufs=2, space="PSUM"))

        # ---- constants, loaded once -------------------------------------
        # w1 + b1 + the first supertile's xT go first so layer 1 of
        # supertile 0 can start while the remaining consts stream in.
        w1_sb = consts.tile([128, KT1, H], dt1)
        nc.sync.dma_start(out=w1_sb, in_=w1_d.rearrange("(a p) h -> p a h", p=128))
        b1_sb = consts.tile([128, HC1], F32)
        nc.sync.dma_start(out=b1_sb, in_=b1_d)
        xt_first = xp.tile([128, KT1, RSUP], dt1, tag="xt")
        nc.sync.dma_start(
            out=xt_first,
            in_=xt_d[:, 0:RSUP].rearrange("(a p) c -> p a c", p=128),
        )
        w2_sb = consts.tile([128, KT2, D], dt2)
        nc.sync.dma_start(out=w2_sb, in_=w2_d.rearrange("(a p) d -> p a d", p=128))
        wg_sb = consts.tile([128, KT1, WGC], BF16)
        nc.sync.dma_start(out=wg_sb, in_=wg_d.rearrange("(a p) h -> p a h", p=128))
        b2_sb = consts.tile([128, DC2], F32)
        nc.sync.dma_start(out=b2_sb, in_=b2_d)
        e1_sb = consts.tile([128, HEADS], F32)
        nc.sync.dma_start(out=e1_sb, in_=e1_d)
        id_sb = consts.tile([65, 65], F32)
        nc.sync.dma_start(out=id_sb, in_=id_d)

        ones_sb = consts.tile([128, 1], BF16)
        nc.vector.memset(ones_sb, 1.0 / D)
        eps_sb = consts.tile([128, 1], F32)
        nc.vector.memset(eps_sb, EPS)

        out_sb = consts.tile([128, nsup * NSUB], F32)

        def e_chain(st, rs4, pt_ps):
            """sigmoid->exp chain for supertile st; runs on ACT/DVE during
            the next supertile's L1 phase.  Returns (e_t, e_b)."""
            e_t = ep.tile([128, NSUB, HEADS], F32, tag="e", bufs=4, name="e_t")
            e_b = ep.tile([128, NSUB, HEADS], BF16, tag="eb", bufs=4, name="e_b")
            for sub in range(NSUB):
                ew = sp.tile([128, HEADS], F32, tag="ew", name="ew")
                nc.scalar.activation(
                    ew, pt_ps[:, sub, 0:HEADS],
                    AF.Exp, scale=rs4[:, sub : sub + 1],
                )
                nc.vector.tensor_tensor(ew, ew, e1_sb, op=OP.mult)
                nc.vector.tensor_scalar(ew, ew, 1.0, None, op0=OP.add)
                sg = sp.tile([128, HEADS], F32, tag="sg", name="sg")
                nc.vector.reciprocal(sg, ew)
                nc.scalar.activation(e_t[:, sub, :], sg, AF.Exp)
                nc.vector.tensor_copy(e_b[:, sub, :], e_t[:, sub, :])
            return e_t, e_b

        def seg_sum(pl):
            """segment sums for supertile st (deps one block old) ->
            reciprocal on DVE.  Returns segr."""
            st0, s_t0, e_b0 = pl
            seg_ps = pseg.tile([128, HEADS], F32, tag="seg", name="seg_ps")
            for sub in range(NSUB):
                nc.tensor.matmul(
                    seg_ps, s_t0[:, sub, :], e_b0[:, sub, :],
                    start=(sub == 0), stop=(sub == NSUB - 1),
                )
            segr = sp.tile([128, HEADS], F32, tag="segr", name="segr")
            nc.vector.tensor_scalar(segr, seg_ps, 1e-30, None, op0=OP.add)
            nc.vector.reciprocal(segr, segr)
            return segr

        def seg_gather(pl):
            """gather + normalize + reduce for supertile st (deps two
            blocks old); writes the output columns."""
            st0, st_t0, e_t0, segr0 = pl
            r_ps = psmall.tile([128, NSUB, HEADS], F32, tag="r_ps", bufs=1,
                               name="r_ps")
            for sub in range(NSUB):
                col = st0 * NSUB + sub
                nc.tensor.matmul(
                    r_ps[:, sub, :], st_t0[:, sub, :], segr0,
                    start=True, stop=True, skip_group_check=True,
                )
                nrm = sp.tile([128, HEADS], F32, tag="nrm", name="nrm")
                nc.vector.tensor_tensor(nrm, e_t0[:, sub, :], r_ps[:, sub, :],
                                        op=OP.mult)
                nc.vector.tensor_reduce(
                    out_sb[:, col : col + 1], nrm,
                    axis=mybir.AxisListType.X, op=OP.add,
                )

        pend_sum: list = []     # (st, s_sb, e_b) awaiting segment-sum
        pend_gat: list = []     # (st, st_sb, e_t, segr) awaiting gather
        for st in range(nsup):
            r0 = st * RSUP
            if st == 0:
                xt_t = xt_first
                s_sb = sdma.tile([128, NSUB, 128], BF16, tag="s", name="s_sb")
                nc.sync.dma_start(out=s_sb, in_=s_d[:, 0])
                st_sb = sdma.tile([128, NSUB, 128], F32, tag="stp", bufs=4,
                                  name="st_sb")
                nc.sync.dma_start(out=st_sb, in_=st_d[:, 0])
            else:
                xt_t, s_sb, st_sb = nxt

            # ---- layer 1: h1T[hc] = relu(W1[:,hc].T @ xT + b1) ----------
            h1_t = hp.tile([128, KT2, RSUP], dt2, tag="h1")
            for hc in range(HC1):
                ps1 = pbig.tile([128, RSUP], F32, tag="pbig")
                if fp8_l1:
                    for kt in range(KT1 // 2):
                        nc.tensor.matmul(
                            ps1,
                            w1_sb[:, 2 * kt : 2 * kt + 2, hc * 128 : (hc + 1) * 128],
                            xt_t[:, 2 * kt : 2 * kt + 2, :],
                            start=(kt == 0),
                            stop=(kt == KT1 // 2 - 1),
                            perf_mode=DR,
                        )
                else:
                    for kt in range(KT1):
                        nc.tensor.matmul(
                            ps1,
                            w1_sb[:, kt, hc * 128 : (hc + 1) * 128],
                            xt_t[:, kt, :],
                            start=(kt == 0),
                            stop=(kt == KT1 - 1),
                        )
                if hc % 8 < 5:
                    nc.scalar.activation(
                        h1_t[:, hc, :], ps1, AF.Relu, bias=b1_sb[:, hc : hc + 1]
                    )
                else:
                    nc.vector.tensor_scalar(
                        h1_t[:, hc, :], ps1, b1_sb[:, hc : hc + 1], 0.0,
                        op0=OP.add, op1=OP.max,
                    )
                if hc == 0 and st + 1 < nsup:
                    # prefetch next supertile inputs while L1 streams
                    r1 = (st + 1) * RSUP
                    xt_n = xp.tile([128, KT1, RSUP], dt1, tag="xt", name="xt_t")
                    nc.sync.dma_start(
                        out=xt_n,
                        in_=xt_d[:, r1 : r1 + RSUP].rearrange(
                            "(a p) c -> p a c", p=128),
                    )
                    s_n = sdma.tile([128, NSUB, 128], BF16, tag="s", name="s_sb")
                    nc.sync.dma_start(out=s_n, in_=s_d[:, st + 1])
                    st_n = sdma.tile([128, NSUB, 128], F32, tag="stp", bufs=4,
                                     name="st_sb")
                    nc.sync.dma_start(out=st_n, in_=st_d[:, st + 1])
                    nxt = (xt_n, s_n, st_n)

            # ---- layer 2 (head projection deferred past the seg MMs) ----
            h2_t = hp.tile([128, DC2, RSUP], BF16, tag="h2")
            sq_t = hp.tile([128, DC2, RSUP], BF16, tag="sq")
            p_ps = pstat.tile([65, RSUP], F32, tag="p")
            for dc in range(DC2):
                ps2 = pbig.tile([128, RSUP], F32, tag="pbig")
                if fp8_l2:
                    for kt in range(KT2 // 2):
                        nc.tensor.matmul(
                            ps2,
                            w2_sb[:, 2 * kt : 2 * kt + 2, dc * 128 : (dc + 1) * 128],
                            h1_t[:, 2 * kt : 2 * kt + 2, :],
                            start=(kt == 0),
                            stop=(kt == KT2 // 2 - 1),
                            perf_mode=DR,
                        )
                else:
                    for kt in range(KT2):
                        nc.tensor.matmul(
                            ps2,
                            w2_sb[:, kt, dc * 128 : (dc + 1) * 128],
                            h1_t[:, kt, :],
                            start=(kt == 0),
                            stop=(kt == KT2 - 1),
                        )
                h2c = h2_t[:, dc, :]
                nc.scalar.activation(
                    h2c, ps2, AF.Relu, bias=b2_sb[:, dc : dc + 1], scale=scale2
                )
                nc.vector.tensor_tensor(sq_t[:, dc, :], h2c, h2c, op=OP.mult)

            # msq tree-sum on DVE (consumed by the ones matmul below)
            t01_sb = hp.tile([128, RSUP], BF16, tag="t01")
            nc.vector.tensor_tensor(t01_sb, sq_t[:, 0, :], sq_t[:, 1, :], op=OP.add)
            t23_sb = hp.tile([128, RSUP], BF16, tag="t23")
            nc.vector.tensor_tensor(t23_sb, sq_t[:, 2, :], sq_t[:, 3, :], op=OP.add)
            sqs_sb = hp.tile([128, RSUP], BF16, tag="sqs")
            nc.vector.tensor_tensor(sqs_sb, t01_sb, t23_sb, op=OP.add)

            # deferred segment MMs (all deps >= 1 block old -> no stalls)
            if pend_sum:
                pl = pend_sum.pop(0)
                pend_gat.append((pl[0], pl[3], pl[4], seg_sum(pl[:3])))
            if pend_gat and len(pend_gat) > 1:
                seg_gather(pend_gat.pop(0))

            # head projection + mean-square reduce into p_ps
            for dc in range(DC2):
                nc.tensor.matmul(
                    p_ps[0:WGC, :], wg_sb[:, dc, :], h2_t[:, dc, :],
                    start=(dc == 0), stop=(dc == DC2 - 1),
                )
            nc.tensor.matmul(p_ps[64:65, :], ones_sb, sqs_sb, start=True,
                             stop=True, skip_group_check=True)

            # ---- stats: one bulk copy, 4 transposes carrying Pc/mu/ms ---
            pv_sb = sp.tile([65, RSUP], F32, tag="pv_sb", bufs=2)
            nc.vector.tensor_copy(pv_sb, p_ps)
            pt_ps = psmall.tile([128, NSUB, 65], F32, tag="pt_ps", bufs=1)
            for sub in range(NSUB):
                nc.tensor.transpose(
                    pt_ps[:, sub, :],
                    pv_sb[:, sub * 128 : (sub + 1) * 128],
                    id_sb,
                )
            # var = ms - mu^2 in the transposed [128, NSUB] layout
            mu_c = sp.tile([128, NSUB], F32, tag="mu_c")
            nc.vector.tensor_copy(mu_c, pt_ps[:, :, 32])
            mu2 = sp.tile([128, NSUB], F32, tag="mu2")
            nc.vector.tensor_tensor(mu2, mu_c, mu_c, op=OP.mult)
            var4 = sp.tile([128, NSUB], F32, tag="var4")
            nc.vector.tensor_tensor(var4, pt_ps[:, :, 64], mu2, op=OP.subtract)
            # rs = 1/std via exp(-0.5*ln(var+eps))  (Rsqrt AF is banned)
            ln4_sb = sp.tile([128, NSUB], F32, tag="ln4_sb")
            nc.scalar.activation(ln4_sb, var4, AF.Ln, bias=eps_sb)
            rs4 = sp.tile([128, NSUB], F32, tag="rs4")
            nc.scalar.activation(rs4, ln4_sb, AF.Exp, scale=-0.5)

            e_t, e_b = e_chain(st, rs4, pt_ps)
            pend_sum.append((st, s_sb, e_b, st_sb, e_t))

        # drain the pipeline
        pl = pend_sum.pop(0)
        pend_gat.append((pl[0], pl[3], pl[4], seg_sum(pl[:3])))
        while pend_gat:
            seg_gather(pend_gat.pop(0))
        nc.sync.dma_start(out=out_d, in_=out_sb)

    nc.finalize()
    return nc


_BUILD_CACHE: dict = {}


def _get_bass(nsup: int, fp8_l1: bool = FP8_L1, fp8_l2: bool = FP8_L2):
    key = (nsup, fp8_l1, fp8_l2)
    if key not in _BUILD_CACHE:
        _BUILD_CACHE[key] = _build_bass(nsup, fp8_l1, fp8_l2)
    return _BUILD_CACHE[key]


def _host_prep(x, row, alpha, W1, b1, W2, b2, ln_g, ln_b, Wa, ba,
               nsup=NSUP, ncores=NCORES):
    """Pack rows into segment-aligned supertiles, build per-core inputs."""
    bf16 = ml_dtypes.bfloat16
    fp8 = ml_dtypes.float8_e4m3
    np1 = fp8 if FP8_L1 else bf16
    np2 = fp8 if FP8_L2 else bf16
    N = x.shape[0]
    row = np.asarray(row).astype(np.int64)

    # segment runs (row is sorted)
    change = np.flatnonzero(np.diff(row)) + 1
    starts = np.concatenate([[0], change])
    ends = np.concatenate([change, [N]])
    lens = ends - starts
    assert lens.max() <= RSUP, "segment longer than a supertile"

    # greedy pack segments into RSUP-row bins
    bin_of_seg = np.empty(len(starts), np.int64)
    nbins = 0
    cur = 0
    for i, ln in enumerate(lens):
        if cur + ln > RSUP:
            nbins += 1
            cur = 0
        bin_of_seg[i] = nbins
        cur += ln
    nbins += 1
    assert nbins <= ncores * nsup, f"{nbins} bins > capacity {ncores * nsup}"

    nr = nsup * RSUP
    # per-core gather index (source row or -1) and local segment rank
    gidx = np.full((ncores, nr), -1, np.int64)
    cloc = np.full((ncores, nr), 127, np.int64)
    seg_rank = np.zeros(len(starts), np.int64)
    # rank of each segment within its bin; position of each segment in bin
    pos_in_bin = np.zeros(len(starts), np.int64)
    cur_bin, cur_pos, cur_rank = -1, 0, 0
    for i in range(len(starts)):
        if bin_of_seg[i] != cur_bin:
            cur_bin, cur_pos, cur_rank = bin_of_seg[i], 0, 0
        pos_in_bin[i] = cur_pos
        seg_rank[i] = cur_rank
        cur_pos += lens[i]
        cur_rank += 1
    assert seg_rank.max() <= 126, "too many segments in one supertile"

    for i in range(len(starts)):
        b = bin_of_seg[i]
        c, stl = divmod(b, nsup)
        base = stl * RSUP + pos_in_bin[i]
        gidx[c, base : base + lens[i]] = np.arange(starts[i], ends[i])
        cloc[c, base : base + lens[i]] = seg_rank[i]

    # x gather + transpose, per core
    x_ext = np.concatenate([x, np.zeros((1, D), np.float32)], 0).astype(np1)
    srcs = np.where(gidx < 0, N, gidx)
    xts = []
    for c in range(ncores):
        xc = x_ext[srcs[c]]                       # [nr, D]
        xts.append(np.ascontiguousarray(xc.T))    # [D, nr]

    # one-hot S [r, j] and S^T [j, r] per subtile, built on host.
    # layout [128, nsup, NSUB, 128]: s[p, st, sub, j] refers to row
    # st*512 + sub*128 + p, local segment id j.
    jj = np.arange(128)
    ss, sts = [], []
    for c in range(ncores):
        cl = cloc[c].reshape(nsup, NSUB, 128)     # [st, sub, p]
        s4 = (cl[:, :, :, None] == jj[None, None, None, :])   # [st,sub,p,j]
        ss.append(np.ascontiguousarray(
            s4.transpose(2, 0, 1, 3)).astype(bf16))           # [p,st,sub,j]
        sts.append(np.ascontiguousarray(
            s4.transpose(3, 0, 1, 2)).astype(np.float32))     # [j,st,sub,p]

    # replicated weights / constants
    alpha_f = float(np.asarray(alpha).reshape(-1)[0])
    sc1 = 8.0 if FP8_L1 else 1.0
    sc2 = 16.0 if FP8_L2 else 1.0
    w1 = np.ascontiguousarray(W1[:D] * sc1).astype(np1)               # [D, H]
    b1_eff = (sc1 * (b1 + alpha_f * W1[D])).astype(np.float32)        # [H]
    b1e = np.ascontiguousarray(b1_eff.reshape(HC1, 128).T)            # [128,16]
    w2 = (W2 * sc2).astype(np2)                                       # [H, D]
    b2t = np.ascontiguousarray(b2.astype(np.float32).reshape(DC2, 128).T)
    wg_f = (ln_g[:, None] * Wa).astype(np.float32)                    # [D, 4]
    s_vec = wg_f.sum(0, keepdims=True)                                # [1, 4]
    wg = np.zeros((D, WGC), np.float32)
    wg[:, 0:HEADS] = -(wg_f - s_vec / D)
    wg[:, 32] = 1.0 / D
    wg = wg.astype(bf16)                                              # [D, 16]
    c1 = (ln_b @ Wa + ba).astype(np.float32)                          # [4]
    e1b = np.broadcast_to(np.exp(-c1), (128, HEADS)).astype(np.float32).copy()
    ident = np.eye(65, dtype=np.float32)

    in_maps = []
    for c in range(ncores):
        in_maps.append({
            "xt": xts[c], "w1": w1, "b1e": b1e, "w2": w2,
            "b2t": b2t, "wg": wg, "e1b": e1b, "ident": ident,
            "sone": ss[c], "sonet": sts[c],
        })
    return in_maps, gidx


def _unshard(results, gidx, N):
    out = np.zeros((N, 1), np.float32)
    for c, res in enumerate(results):
        vals = np.ascontiguousarray(res["out"].T).reshape(-1, NSUB, 128)
        vals = vals.reshape(-1)  # position order (st, sub, p)
        m = gidx[c] >= 0
        out[gidx[c][m], 0] = vals[m]
    return out * (1.0 / HEADS)


def kernel(x, row, alpha, W1, b1, W2, b2, ln_g, ln_b, Wa, ba, **_kw):
    x = np.asarray(x, np.float32)
    in_maps, gidx = _host_prep(
        x, row, alpha,
        np.asarray(W1, np.float32), np.asarray(b1, np.float32),
        np.asarray(W2, np.float32), np.asarray(b2, np.float32),
        np.asarray(ln_g, np.float32), np.asarray(ln_b, np.float32),
        np.asarray(Wa, np.float32), np.asarray(ba, np.float32),
    )
    nc = _get_bass(NSUP)
    res = bass_utils.run_bass_kernel_spmd(
        nc, in_maps, core_ids=list(range(NCORES))
    )
    return _unshard(res.results, gidx, x.shape[0])


# revision 12
# speedup vs baseline: 1.3573x; 1.1528x over previous
"""Trainium2 Bass kernel for nn_EnhancedAttentionLayer (segment softmax MLP).

Host contract: kernel(**inputs) takes the FULL unsharded inputs from
setup_inputs() and returns the FULL [N, 1] float32 output.

Strategy
--------
Math:  out = mean_heads( softmax_per_segment( sigmoid( Wa-head of
       LN( relu( relu([x, alpha] @ W1 + b1) @ W2 + b2 ) ) ) ) )

Device layout ("transposed chain", weights stationary as lhsT):
  xT [D, rows]  --W1-->  h1T [H, rows]  --W2-->  h2T [D, rows]
Matmul contracts over the partition dim, so feeding x transposed keeps
every matmul's moving operand at N=512 free-dim (full PE efficiency) with
zero on-device transposes of activations in the main chain.

fp8 DoubleRow: the two big GEMMs can run in fp8e4 (TRN E4M3, max 240)
with perf_mode=DoubleRow, which packs 2 k-subtiles per matmul (2 fp8
MACs/cell/cycle) for ~1.8x PE throughput.  Weights are pre-scaled by a
power of two (W1*8, W2*16) so their uniform(-1/sqrt(K)) entries escape
the e4m3 subnormal range; the scale is divided back out in the (free)
activation-scale of the following relu.  h1 is stored scaled by 8 (no
extra op) and the combined 1/128 lands in L2's activation scale.

Folds (host-side precompute):
  - alpha column:  b1_eff = b1 + alpha * W1[D]   (concat removed)
  - LN scale/bias g,b folded into the head projection; the rank-1
    -s*mu LN term is folded into the projection weights themselves:
      raw_att[r,a] = rs[r] * Pc[a,r] + c1[a],
      Pc = (g.Wa - s/D)^T @ h2T,  s = sum_d g*Wa,  c1 = b@Wa + ba
    wg is stored NEGATED so exp(-raw) = exp(rs * (-Pc)) * exp(-c1)
    needs no negation of rs on device.  wg col 4 = +1/D gives mu.
  - segment max subtraction dropped: z = sigmoid(.) in (0,1), so
    softmax = exp(z)/segsum(exp(z)) is numerically safe without it.

Segment softmax on device: rows are pre-packed on host so every 512-row
supertile is segment-aligned (a segment never straddles supertiles) with
<=127 segments per supertile.  Per 128-row subtile the one-hot S[r, j] =
(j == local_seg[r]) and its transpose are built on HOST and DMAd in
(bf16); segment sums are S^T @ E (PE matmul), the gather back is
S @ recip(segsum) (PE matmul with the DMAd S^T as stationary).

Sharding: 8 cores data-parallel over supertiles (segments never cross
cores).  Weights replicated.  SPMD: one Bass program, per-core inputs.
"""

import sys

sys.path.insert(0, "/opt/trn_rl_repo")

from contextlib import ExitStack

import ml_dtypes
import numpy as np

import concourse.bass as bass
import concourse.tile as tile
from concourse import bacc, bass_utils, mybir

BF16 = mybir.dt.bfloat16
FP8 = mybir.dt.float8e4
F32 = mybir.dt.float32
AF = mybir.ActivationFunctionType
OP = mybir.AluOpType
DR = mybir.MatmulPerfMode.DoubleRow

D = 512
H = 2048
HEADS = 4
RSUP = 512          # rows per supertile
NSUP = 25           # supertiles per core
NCORES = 8
EPS = 1e-5
KT1 = D // 128      # 4  k-tiles for layer 1
KT2 = H // 128      # 16 k-tiles for layer 2
HC1 = H // 128      # 16 output chunks for layer 1
DC2 = D // 128      # 4  output chunks for layer 2
NSUB = RSUP // 128  # 4  subtiles per supertile
WGC = 33            # head-projection width (cols 0-3 heads, col 32 mu)

FP8_L1 = True       # layer 1 in fp8 DoubleRow
FP8_L2 = False      # layer 2 in fp8 DoubleRow
SC1 = 8.0 if FP8_L1 else 1.0      # W1 pre-scale (h1 stored at SC1*h1)
SC2 = 16.0 if FP8_L2 else 1.0     # W2 pre-scale
SCALE2 = 1.0 / (SC1 * SC2)        # L2 activation scale


def _build_bass(nsup: int, fp8_l1: bool, fp8_l2: bool):
    """Build the SPMD single-core Bass program for `nsup` supertiles."""
    nc = bacc.Bacc(
        "TRN2", target_bir_lowering=False, debug=False, enable_asserts=False
    )
    nr = nsup * RSUP
    dt1 = FP8 if fp8_l1 else BF16   # xt / w1 dtype
    dt2 = FP8 if fp8_l2 else BF16   # h1 / w2 dtype
    scale2 = 1.0 / ((8.0 if fp8_l1 else 1.0) * (16.0 if fp8_l2 else 1.0))

    xt_d = nc.dram_tensor("xt", [D, nr], dt1, kind="ExternalInput").ap()
    w1_d = nc.dram_tensor("w1", [D, H], dt1, kind="ExternalInput").ap()
    b1_d = nc.dram_tensor("b1e", [128, HC1], F32, kind="ExternalInput").ap()
    w2_d = nc.dram_tensor("w2", [H, D], dt2, kind="ExternalInput").ap()
    b2_d = nc.dram_tensor("b2t", [128, DC2], F32, kind="ExternalInput").ap()
    wg_d = nc.dram_tensor("wg", [D, WGC], BF16, kind="ExternalInput").ap()
    e1_d = nc.dram_tensor("e1b", [128, NSUB * HEADS], F32, kind="ExternalInput").ap()
    id_d = nc.dram_tensor("ident", [65, 65], F32, kind="ExternalInput").ap()
    s_d = nc.dram_tensor(
        "sone", [128, nsup, NSUB, 128], BF16, kind="ExternalInput"
    ).ap()
    st_d = nc.dram_tensor(
        "sonet", [128, nsup, NSUB, 128], F32, kind="ExternalInput"
    ).ap()
    out_d = nc.dram_tensor("out", [128, nsup * NSUB], F32, kind="ExternalOutput").ap()

    with tile.TileContext(nc) as tc, ExitStack() as ctx:
        consts = ctx.enter_context(tc.tile_pool(name="consts", bufs=1))
        xp = ctx.enter_context(tc.tile_pool(name="xp", bufs=2))
        hp = ctx.enter_context(tc.tile_pool(name="hp", bufs=2))
        ep = ctx.enter_context(tc.tile_pool(name="ep", bufs=4))
        sp = ctx.enter_context(tc.tile_pool(name="sp", bufs=3))
        sdma = ctx.enter_context(tc.tile_pool(name="sdma", bufs=4))
        pbig = ctx.enter_context(tc.tile_pool(name="pbig", bufs=4, space="PSUM"))
        pstat = ctx.enter_context(tc.tile_pool(name="pstat", bufs=1, space="PSUM"))
        psmall = ctx.enter_context(tc.tile_pool(name="psmall", bufs=1, space="PSUM"))

        # ---- constants, loaded once -------------------------------------
        # w1 + b1 + the first supertile's xT go first so layer 1 of
        # supertile 0 can start while the remaining consts stream in.
        w1_sb = consts.tile([128, KT1, H], dt1)
        nc.sync.dma_start(out=w1_sb, in_=w1_d.rearrange("(a p) h -> p a h", p=128))
        b1_sb = consts.tile([128, HC1], F32)
        nc.sync.dma_start(out=b1_sb, in_=b1_d)
        xt_first = xp.tile([128, KT1, RSUP], dt1, tag="xt")
        nc.sync.dma_start(
            out=xt_first,
            in_=xt_d[:, 0:RSUP].rearrange("(a p) c -> p a c", p=128),
        )
        w2_sb = consts.tile([128, KT2, D], dt2)
        nc.sync.dma_start(out=w2_sb, in_=w2_d.rearrange("(a p) d -> p a d", p=128))
        wg_sb = consts.tile([128, KT1, WGC], BF16)
        nc.sync.dma_start(out=wg_sb, in_=wg_d.rearrange("(a p) h -> p a h", p=128))
        b2_sb = consts.tile([128, DC2], F32)
        nc.sync.dma_start(out=b2_sb, in_=b2_d)
        e1_sb = consts.tile([128, NSUB * HEADS], F32)
        nc.sync.dma_start(out=e1_sb, in_=e1_d)
        id_sb = consts.tile([65, 65], F32)
        nc.sync.dma_start(out=id_sb, in_=id_d)

        ones_sb = consts.tile([128, 1], BF16)
        nc.vector.memset(ones_sb, 1.0 / D)
        eps_sb = consts.tile([128, 1], F32)
        nc.vector.memset(eps_sb, EPS)

        out_sb = consts.tile([128, nsup * NSUB], F32)

        def tail(tl):
            """head projection + mean-square + LN stats + sigmoid/exp chain
            for supertile st0, emitted interleaved into the NEXT block's L2
            phase (all inputs one block old).  Generator: each yield returns
            control so PE-heavy pieces land between L2 dc-groups."""
            st0, h2_0, sq_0, s_0, st_0 = tl
            p_ps = pstat.tile([65, RSUP], F32, tag="p")
            for dc in range(DC2):
                nc.tensor.matmul(
                    p_ps[0:WGC, :], wg_sb[:, dc, :], h2_0[:, dc, :],
                    start=(dc == 0), stop=(dc == DC2 - 1),
                )
            for dc in range(DC2):
                nc.tensor.matmul(
                    p_ps[64:65, :], ones_sb, sq_0[:, dc, :],
                    start=(dc == 0), stop=(dc == DC2 - 1),
                    skip_group_check=True,
                )
            pv_sb = sp.tile([65, RSUP], F32, tag="pv_sb", bufs=2)
            nc.vector.tensor_copy(pv_sb, p_ps)
            yield
            pt_ps = psmall.tile([128, NSUB, 65], F32, tag="pp", name="pt_ps")
            for sub in range(NSUB):
                nc.tensor.transpose(
                    pt_ps[:, sub, :],
                    pv_sb[:, sub * 128 : (sub + 1) * 128],
                    id_sb,
                )
            # var = ms - mu^2 in the transposed [128, NSUB] layout
            mu_c = sp.tile([128, NSUB], F32, tag="mu_c")
            nc.vector.tensor_copy(mu_c, pt_ps[:, :, 32])
            mu2 = sp.tile([128, NSUB], F32, tag="mu2")
            nc.vector.tensor_tensor(mu2, mu_c, mu_c, op=OP.mult)
            var4 = sp.tile([128, NSUB], F32, tag="var4")
            nc.vector.tensor_tensor(var4, pt_ps[:, :, 64], mu2, op=OP.subtract)
            # rs = 1/std via exp(-0.5*ln(var+eps))  (Rsqrt AF is banned)
            ln4_sb = sp.tile([128, NSUB], F32, tag="ln4_sb")
            nc.scalar.activation(ln4_sb, var4, AF.Ln, bias=eps_sb)
            rs4 = sp.tile([128, NSUB], F32, tag="rs4", bufs=2)
            nc.scalar.activation(rs4, ln4_sb, AF.Exp, scale=-0.5)
            yield
            # sigmoid -> exp chain, batched over all 4 subtiles
            ew = sp.tile([128, NSUB, HEADS], F32, tag="ew", name="ew")
            for sub in range(NSUB):
                nc.scalar.activation(
                    ew[:, sub, :], pt_ps[:, sub, 0:HEADS],
                    AF.Exp, scale=rs4[:, sub : sub + 1],
                )
            nc.vector.tensor_tensor(ew, ew, e1_sb, op=OP.mult)
            nc.vector.tensor_scalar(ew, ew, 1.0, None, op0=OP.add)
            sg = sp.tile([128, NSUB, HEADS], F32, tag="sg", name="sg")
            nc.vector.reciprocal(sg, ew)
            e_t = ep.tile([128, NSUB, HEADS], F32, tag="e", name="e_t")
            nc.scalar.activation(e_t, sg, AF.Exp)
            e_b = ep.tile([128, NSUB, HEADS], BF16, tag="eb", name="e_b")
            nc.vector.tensor_copy(e_b, e_t)
            pend_sum.append((st0, s_0, e_b, st_0, e_t))

        def seg_sum(pl):
            """segment sums for supertile st (deps one block old) ->
            reciprocal on DVE."""
            st0, s_t0, e_b0, st_t0, e_t0 = pl
            seg_ps = psmall.tile([128, HEADS], F32, tag="pp", name="seg_ps")
            for sub in range(NSUB):
                nc.tensor.matmul(
                    seg_ps, s_t0[:, sub, :], e_b0[:, sub, :],
                    start=(sub == 0), stop=(sub == NSUB - 1),
                )
            segr = sp.tile([128, HEADS], F32, tag="segr", name="segr")
            nc.vector.tensor_scalar(segr, seg_ps, 1e-30, None, op0=OP.add)
            nc.vector.reciprocal(segr, segr)
            pend_gat.append((st0, st_t0, e_t0, segr))

        def seg_gather(pl):
            """gather + normalize + reduce for supertile st (deps two
            blocks old); writes the output columns."""
            st0, st_t0, e_t0, segr0 = pl
            r_ps = psmall.tile([128, NSUB, HEADS], F32, tag="pp", name="r_ps")
            for sub in range(NSUB):
                col = st0 * NSUB + sub
                nc.tensor.matmul(
                    r_ps[:, sub, :], st_t0[:, sub, :], segr0,
                    start=True, stop=True, skip_group_check=True,
                )
                nrm = sp.tile([128, HEADS], F32, tag="nrm", name="nrm")
                nc.vector.tensor_tensor(nrm, e_t0[:, sub, :], r_ps[:, sub, :],
                                        op=OP.mult)
                nc.vector.tensor_reduce(
                    out_sb[:, col : col + 1], nrm,
                    axis=mybir.AxisListType.X, op=OP.add,
                )

        pend_sum: list = []   # supertiles awaiting segment-sum
        pend_gat: list = []   # supertiles awaiting gather
        pend_tail = None      # supertile awaiting the stats/sigmoid tail
        nxt = None
        for st in range(nsup):
            r0 = st * RSUP
            if st == 0:
                xt_t = xt_first
                s_sb = sdma.tile([128, NSUB, 128], BF16, tag="s", name="s_sb")
                nc.sync.dma_start(out=s_sb, in_=s_d[:, 0])
                st_sb = sdma.tile([128, NSUB, 128], F32, tag="stp", bufs=5,
                                  name="st_sb")
                nc.sync.dma_start(out=st_sb, in_=st_d[:, 0])
            else:
                xt_t, s_sb, st_sb = nxt

            # ---- phase A: layer 1, interleaved with deferred seg MMs ----
            h1_t = hp.tile([128, KT2, RSUP], dt2, tag="h1")
            for hc in range(HC1):
                ps1 = pbig.tile([128, RSUP], F32, tag="l1", bufs=4)
                if fp8_l1:
                    for kt in range(KT1 // 2):
                        nc.tensor.matmul(
                            ps1,
                            w1_sb[:, 2 * kt : 2 * kt + 2, hc * 128 : (hc + 1) * 128],
                            xt_t[:, 2 * kt : 2 * kt + 2, :],
                            start=(kt == 0),
                            stop=(kt == KT1 // 2 - 1),
                            perf_mode=DR,
                        )
                else:
                    for kt in range(KT1):
                        nc.tensor.matmul(
                            ps1,
                            w1_sb[:, kt, hc * 128 : (hc + 1) * 128],
                            xt_t[:, kt, :],
                            start=(kt == 0),
                            stop=(kt == KT1 - 1),
                        )
                if hc % 2 == 0:
                    nc.scalar.activation(
                        h1_t[:, hc, :], ps1, AF.Relu, bias=b1_sb[:, hc : hc + 1]
                    )
                else:
                    nc.vector.tensor_scalar(
                        h1_t[:, hc, :], ps1, b1_sb[:, hc : hc + 1], 0.0,
                        op0=OP.add, op1=OP.max,
                    )
                if hc == 0 and st + 1 < nsup:
                    # prefetch next supertile inputs while L1 streams
                    r1 = (st + 1) * RSUP
                    xt_n = xp.tile([128, KT1, RSUP], dt1, tag="xt", name="xt_t")
                    nc.sync.dma_start(
                        out=xt_n,
                        in_=xt_d[:, r1 : r1 + RSUP].rearrange(
                            "(a p) c -> p a c", p=128),
                    )
                    s_n = sdma.tile([128, NSUB, 128], BF16, tag="s", name="s_sb")
                    nc.sync.dma_start(out=s_n, in_=s_d[:, st + 1])
                    st_n = sdma.tile([128, NSUB, 128], F32, tag="stp", bufs=5,
                                     name="st_sb")
                    nc.sync.dma_start(out=st_n, in_=st_d[:, st + 1])
                    nxt = (xt_n, s_n, st_n)
                if hc == 1 and pend_sum:
                    seg_sum(pend_sum.pop(0))
                if hc == 8 and len(pend_gat) > 1:
                    seg_gather(pend_gat.pop(0))

            # ---- phase B: layer 2, interleaved with previous tile's tail -
            tgen = tail(pend_tail) if pend_tail is not None else None
            h2_t = hp.tile([128, DC2, RSUP], BF16, tag="h2")
            sq_t = hp.tile([128, DC2, RSUP], BF16, tag="sq")
            for dc in range(DC2):
                ps2 = pbig.tile([128, RSUP], F32, tag="l2", bufs=2)
                if fp8_l2:
                    for kt in range(KT2 // 2):
                        nc.tensor.matmul(
                            ps2,
                            w2_sb[:, 2 * kt : 2 * kt + 2, dc * 128 : (dc + 1) * 128],
                            h1_t[:, 2 * kt : 2 * kt + 2, :],
                            start=(kt == 0),
                            stop=(kt == KT2 // 2 - 1),
                            perf_mode=DR,
                        )
                else:
                    for kt in range(KT2):
                        nc.tensor.matmul(
                            ps2,
                            w2_sb[:, kt, dc * 128 : (dc + 1) * 128],
                            h1_t[:, kt, :],
                            start=(kt == 0),
                            stop=(kt == KT2 - 1),
                        )
                h2c = h2_t[:, dc, :]
                nc.scalar.activation(
                    h2c, ps2, AF.Relu, bias=b2_sb[:, dc : dc + 1], scale=scale2
                )
                nc.gpsimd.tensor_tensor(sq_t[:, dc, :], h2c, h2c, op=OP.mult)
                if tgen is not None and dc < DC2 - 1:
                    next(tgen, None)
            if tgen is not None:
                for _ in tgen:
                    pass
            pend_tail = (st, h2_t, sq_t, s_sb, st_sb)

        # drain the pipeline
        for _ in tail(pend_tail):
            pass
        while pend_sum or pend_gat:
            if pend_sum:
                seg_sum(pend_sum.pop(0))
            seg_gather(pend_gat.pop(0))
        nc.sync.dma_start(out=out_d, in_=out_sb)

    nc.finalize()
    return nc


_BUILD_CACHE: dict = {}


def _get_bass(nsup: int, fp8_l1: bool = FP8_L1, fp8_l2: bool = FP8_L2):
    key = (nsup, fp8_l1, fp8_l2)
    if key not in _BUILD_CACHE:
        _BUILD_CACHE[key] = _build_bass(nsup, fp8_l1, fp8_l2)
    return _BUILD_CACHE[key]


def _host_prep(x, row, alpha, W1, b1, W2, b2, ln_g, ln_b, Wa, ba,
               nsup=NSUP, ncores=NCORES):
    """Pack rows into segment-aligned supertiles, build per-core inputs."""
    bf16 = ml_dtypes.bfloat16
    fp8 = ml_dtypes.float8_e4m3
    np1 = fp8 if FP8_L1 else bf16
    np2 = fp8 if FP8_L2 else bf16
    N = x.shape[0]
    row = np.asarray(row).astype(np.int64)

    # segment runs (row is sorted)
    change = np.flatnonzero(np.diff(row)) + 1
    starts = np.concatenate([[0], change])
    ends = np.concatenate([change, [N]])
    lens = ends - starts
    assert lens.max() <= RSUP, "segment longer than a supertile"

    # greedy pack segments into RSUP-row bins
    bin_of_seg = np.empty(len(starts), np.int64)
    nbins = 0
    cur = 0
    for i, ln in enumerate(lens):
        if cur + ln > RSUP:
            nbins += 1
            cur = 0
        bin_of_seg[i] = nbins
        cur += ln
    nbins += 1
    assert nbins <= ncores * nsup, f"{nbins} bins > capacity {ncores * nsup}"

    nr = nsup * RSUP
    # per-core gather index (source row or -1) and local segment rank
    gidx = np.full((ncores, nr), -1, np.int64)
    cloc = np.full((ncores, nr), 127, np.int64)
    seg_rank = np.zeros(len(starts), np.int64)
    # rank of each segment within its bin; position of each segment in bin
    pos_in_bin = np.zeros(len(starts), np.int64)
    cur_bin, cur_pos, cur_rank = -1, 0, 0
    for i in range(len(starts)):
        if bin_of_seg[i] != cur_bin:
            cur_bin, cur_pos, cur_rank = bin_of_seg[i], 0, 0
        pos_in_bin[i] = cur_pos
        seg_rank[i] = cur_rank
        cur_pos += lens[i]
        cur_rank += 1
    assert seg_rank.max() <= 126, "too many segments in one supertile"

    for i in range(len(starts)):
        b = bin_of_seg[i]
        c, stl = divmod(b, nsup)
        base = stl * RSUP + pos_in_bin[i]
        gidx[c, base : base + lens[i]] = np.arange(starts[i], ends[i])
        cloc[c, base : base + lens[i]] = seg_rank[i]

    # x gather + transpose, per core
    x_ext = np.concatenate([x, np.zeros((1, D), np.float32)], 0).astype(np1)
    srcs = np.where(gidx < 0, N, gidx)
    xts = []
    for c in range(ncores):
        xc = x_ext[srcs[c]]                       # [nr, D]
        xts.append(np.ascontiguousarray(xc.T))    # [D, nr]

    # one-hot S [r, j] and S^T [j, r] per subtile, built on host.
    # layout [128, nsup, NSUB, 128]: s[p, st, sub, j] refers to row
    # st*512 + sub*128 + p, local segment id j.
    jj = np.arange(128)
    ss, sts = [], []
    for c in range(ncores):
        cl = cloc[c].reshape(nsup, NSUB, 128)     # [st, sub, p]
        s4 = (cl[:, :, :, None] == jj[None, None, None, :])   # [st,sub,p,j]
        ss.append(np.ascontiguousarray(
            s4.transpose(2, 0, 1, 3)).astype(bf16))           # [p,st,sub,j]
        sts.append(np.ascontiguousarray(
            s4.transpose(3, 0, 1, 2)).astype(np.float32))     # [j,st,sub,p]

    # replicated weights / constants
    alpha_f = float(np.asarray(alpha).reshape(-1)[0])
    sc1 = 8.0 if FP8_L1 else 1.0
    sc2 = 16.0 if FP8_L2 else 1.0
    w1 = np.ascontiguousarray(W1[:D] * sc1).astype(np1)               # [D, H]
    b1_eff = (sc1 * (b1 + alpha_f * W1[D])).astype(np.float32)        # [H]
    b1e = np.ascontiguousarray(b1_eff.reshape(HC1, 128).T)            # [128,16]
    w2 = (W2 * sc2).astype(np2)                                       # [H, D]
    b2t = np.ascontiguousarray(b2.astype(np.float32).reshape(DC2, 128).T)
    wg_f = (ln_g[:, None] * Wa).astype(np.float32)                    # [D, 4]
    s_vec = wg_f.sum(0, keepdims=True)                                # [1, 4]
    wg = np.zeros((D, WGC), np.float32)
    wg[:, 0:HEADS] = -(wg_f - s_vec / D)
    wg[:, 32] = 1.0 / D
    wg = wg.astype(bf16)                                              # [D, 16]
    c1 = (ln_b @ Wa + ba).astype(np.float32)                          # [4]
    e1b = np.broadcast_to(np.tile(np.exp(-c1), NSUB), (128, NSUB * HEADS)).astype(np.float32).copy()
    ident = np.eye(65, dtype=np.float32)

    in_maps = []
    for c in range(ncores):
        in_maps.append({
            "xt": xts[c], "w1": w1, "b1e": b1e, "w2": w2,
            "b2t": b2t, "wg": wg, "e1b": e1b, "ident": ident,
            "sone": ss[c], "sonet": sts[c],
        })
    return in_maps, gidx


def _unshard(results, gidx, N):
    out = np.zeros((N, 1), np.float32)
    for c, res in enumerate(results):
        vals = np.ascontiguousarray(res["out"].T).reshape(-1, NSUB, 128)
        vals = vals.reshape(-1)  # position order (st, sub, p)
        m = gidx[c] >= 0
        out[gidx[c][m], 0] = vals[m]
    return out * (1.0 / HEADS)


def kernel(x, row, alpha, W1, b1, W2, b2, ln_g, ln_b, Wa, ba, **_kw):
    x = np.asarray(x, np.float32)
    in_maps, gidx = _host_prep(
        x, row, alpha,
        np.asarray(W1, np.float32), np.asarray(b1, np.float32),
        np.asarray(W2, np.float32), np.asarray(b2, np.float32),
        np.asarray(ln_g, np.float32), np.asarray(ln_b, np.float32),
        np.asarray(Wa, np.float32), np.asarray(ba, np.float32),
    )
    nc = _get_bass(NSUP)
    res = bass_utils.run_bass_kernel_spmd(
        nc, in_maps, core_ids=list(range(NCORES))
    )
    return _unshard(res.results, gidx, x.shape[0])
